# revision 16
# baseline (speedup 1.0000x reference)
"""Trainium2 Bass kernel for gated multi-head attention + residual + LayerNorm.

Problem (nn_CNP_5669356834854):
    B=2, L=2048, D=1024, H=16, DK=DV=64
    Q = q@wq.T+bq; K = k@wk.T+bk; V = v@wv.T+bv   (per-head split)
    attn = softmax((Q K^T / sqrt(DK)) * k_gate  [masked])
    out = LayerNorm(attn @ V @ wo.T + bo + q)

Sharding: 8 cores = (batch b) x (head-group hg, 4 heads).  L1 computes
UNNORMALIZED per-head attention outputs O^T plus softmax denominators
(ones-augmented V).  The host normalizes O (cheap elementwise) while
resharding; L2 shards (batch, 512-row chunk): output projection +
residual + LayerNorm.

L1 structure (all in transposed "T-space", no on-chip transposes):
  - 8 blocks = (pr in 2) x (qc in 4): each block covers 2 heads x 512 lq
    over 16 lk-tiles -> 128 global ticks.
  - per tick: S-tile [128, 2hp x 512] f32 in a 3-slot PSUM ring (6 banks)
    -> the DVE gate-multiply (one FD=1024 op/tick) streams back-to-back,
    fully decoupled from the PE.
  - exp on ACT over 4-tick quads (FD=4096).
  - O accumulation [128,512] per head = 2 PSUM banks total, block-serial.
  - software pipeline: back(t-PIPE) trails front(t) across block bounds.
  - every 8th tick's gate-multiply is offloaded (ACT copies S->bf16, the
    idle GpSimd engine multiplies) to balance DVE vs ACT.
  - dual DMA: Sync HWDGE queue (wk, x_k, x_v, gate slabs) in parallel with
    GpSimd SWDGE queue (wq, wv, x_q, outputs).
  - host-packed contiguous gate slabs (256KB per tick); PE warm-up matmuls.
"""

import numpy as np
import ml_dtypes

import concourse.bacc as bacc
import concourse.tile as tile
from concourse import mybir
from concourse.bass_utils import run_bass_kernel_spmd

B, L, D, H, DK, DV = 2, 2048, 1024, 16, 64, 64
EPS = 1e-5
NCORE = 8
HPC = 4  # heads per core
NKC = D // 128  # 8 contraction chunks
NLKT = 16  # lk tiles
CH = 512  # L2 row-chunk per core
MPC = HPC * DK  # 256 projected rows per core
QC = 512  # lq per block
EXP_BIAS = -20.0

F32 = mybir.dt.float32
BF16 = mybir.dt.bfloat16
NPBF16 = ml_dtypes.bfloat16
AF = mybir.ActivationFunctionType

PIPE = 8  # O-matmul lag in global ticks (multiple of 4 not required)
OFF_MOD = 8  # offload every OFF_MODth tick (phase OFF_PHASE) to GpSimd
OFF_PHASE = 2
N_WARM = 12


def _bf(x):
    return np.ascontiguousarray(x).astype(NPBF16)


def _kc_layout(a):
    """[D, N] -> [128, NKC, N] with row r = kc*128+p  ->  [p, kc, :]."""
    d, n = a.shape
    assert d == NKC * 128
    return np.ascontiguousarray(a.reshape(NKC, 128, n).transpose(1, 0, 2))


def build_l1(masked: bool, use_bq: bool, use_bk: bool, use_bv: bool):
    nc = bacc.Bacc("TRN2", target_bir_lowering=False)

    qT = nc.declare_dram_parameter("qT", [128, NKC, L], BF16, isOutput=False)
    kT = nc.declare_dram_parameter("kT", [128, NKC, L], BF16, isOutput=False)
    vT = nc.declare_dram_parameter("vT", [128, NKC, L], BF16, isOutput=False)
    wqT = nc.declare_dram_parameter("wqT", [128, NKC, MPC], BF16, isOutput=False)
    wkT = nc.declare_dram_parameter("wkT", [128, NKC, MPC], BF16, isOutput=False)
    wvT = nc.declare_dram_parameter("wvT", [128, NKC, MPC], BF16, isOutput=False)
    # host-packed gate: gPK[pr, qc, lkt, p, hp*512 + i]
    gPK = nc.declare_dram_parameter(
        "gPK", [2, 4, NLKT, 128, 2 * QC], BF16, isOutput=False
    )
    if use_bq:
        bqP = nc.declare_dram_parameter("bqP", [128, 2], F32, isOutput=False)
    if use_bk:
        bkP = nc.declare_dram_parameter("bkP", [128, 2], F32, isOutput=False)
    if use_bv:
        bvR = nc.declare_dram_parameter("bvR", [1, MPC], F32, isOutput=False)
    if masked:
        mbT = nc.declare_dram_parameter("mbT", [L, L], BF16, isOutput=False)
    # unnormalized O (rows 0:64 per hp) + denominator (row 64)
    oU = nc.declare_dram_parameter("oU", [2, 4, 65, 2, QC], BF16, isOutput=True)

    with tile.TileContext(nc) as tc:
        with (
            tc.tile_pool(name="xs", bufs=3) as xs,
            tc.tile_pool(name="ws", bufs=1) as ws,
            tc.tile_pool(name="qk", bufs=1) as qk,
            tc.tile_pool(name="gp", bufs=8) as gp,
            tc.tile_pool(name="tp", bufs=2) as tp,
            tc.tile_pool(name="pp", bufs=3) as pp,
            tc.tile_pool(name="sb8", bufs=2) as sb8,
            tc.tile_pool(name="op", bufs=2) as opl,
            tc.tile_pool(name="ps_s", bufs=3, space="PSUM") as ps_s,
            tc.tile_pool(name="ps_o", bufs=2, space="PSUM") as ps_o,
        ):
            # ---- DMA streams ----
            # sync (HWDGE): wk, x_k, x_v, then gate slabs (emitted in fronts)
            wk_sb = ws.tile([128, NKC, MPC], BF16, tag="wk")
            nc.sync.dma_start(out=wk_sb, in_=wkT[:, :, :])
            x_k = xs.tile([128, NKC, L], BF16, tag="x", name="x_k")
            nc.sync.dma_start(out=x_k, in_=kT[:, :, :])
            x_v = xs.tile([128, NKC, L], BF16, tag="x", name="x_v")
            nc.sync.dma_start(out=x_v, in_=vT[:, :, :])
            # gpsimd (SWDGE): wq, wv, x_q, then oU outputs
            wq_sb = ws.tile([128, NKC, MPC], BF16, tag="wq")
            nc.gpsimd.dma_start(out=wq_sb, in_=wqT[:, :, :])
            wv_sb = ws.tile([128, NKC, MPC], BF16, tag="wv")
            nc.gpsimd.dma_start(out=wv_sb, in_=wvT[:, :, :])
            x_q = xs.tile([128, NKC, L], BF16, tag="x", name="x_q")
            nc.gpsimd.dma_start(out=x_q, in_=qT[:, :, :])

            QT = qk.tile([128, 2, L], BF16, tag="qt")
            KT = qk.tile([128, 2, L], BF16, tag="kt")
            Vaug = qk.tile([128, NLKT, HPC, 128], BF16, tag="va")
            nc.vector.memset(Vaug[:, :, :, 64:128], 1.0)
            ebias = ws.tile([128, 1], F32, tag="eb")
            nc.vector.memset(ebias, EXP_BIAS)

            bias_tiles = {}
            if use_bq:
                bq_sb = ws.tile([128, 2], F32, tag="bq")
                nc.sync.dma_start(out=bq_sb, in_=bqP[:, :])
                bias_tiles["q"] = bq_sb
            if use_bk:
                bk_sb = ws.tile([128, 2], F32, tag="bk")
                nc.sync.dma_start(out=bk_sb, in_=bkP[:, :])
                bias_tiles["k"] = bk_sb
            if use_bv:
                bv_sb = ws.tile([128, MPC], F32, tag="bv")
                nc.sync.dma_start(out=bv_sb, in_=bvR.ap().to_broadcast([128, MPC]))
                bias_tiles["v"] = bv_sb

            # ---- PE warm-up (on wq, discarded) ----
            warm = ps_s.tile([128, 2 * 512], F32, tag="s", name="warm")
            for i in range(N_WARM):
                nc.tensor.matmul(
                    warm[:, 0:MPC],
                    lhsT=wq_sb[:, 0, 0:128],
                    rhs=wq_sb[:, 0, :],
                    start=True,
                    stop=True,
                    skip_group_check=True,
                )

            # ---- projections ----
            def emit_qk_proj(name, x_sb, w_sb, dst, mt, lqh):
                ps = ps_s.tile(
                    [128, 2 * 512], F32, tag="s", name=f"pj_{name}{mt}{lqh}"
                )
                for c in range(2):
                    lo = lqh * 1024 + c * 512
                    for kc in range(NKC):
                        nc.tensor.matmul(
                            ps[:, c * 512 : (c + 1) * 512],
                            lhsT=w_sb[:, kc, mt * 128 : (mt + 1) * 128],
                            rhs=x_sb[:, kc, lo : lo + 512],
                            start=(kc == 0),
                            stop=(kc == NKC - 1),
                        )
                if name in bias_tiles:
                    nc.vector.tensor_scalar_add(
                        out=dst[:, mt, lqh * 1024 : (lqh + 1) * 1024],
                        in0=ps,
                        scalar1=bias_tiles[name][:, mt : mt + 1],
                    )
                else:
                    nc.scalar.copy(
                        out=dst[:, mt, lqh * 1024 : (lqh + 1) * 1024], in_=ps
                    )

            def emit_v_lkt(lkt):
                ps = ps_o.tile([128, MPC], F32, tag="o", name="pj_v")
                for kc in range(NKC):
                    nc.tensor.matmul(
                        ps,
                        lhsT=x_v[:, kc, lkt * 128 : (lkt + 1) * 128],
                        rhs=wv_sb[:, kc, :],
                        start=(kc == 0),
                        stop=(kc == NKC - 1),
                    )
                psr = ps.rearrange("p (h d) -> p h d", h=HPC)
                if "v" in bias_tiles:
                    nc.vector.tensor_add(
                        out=Vaug[:, lkt, :, 0:64],
                        in0=psr,
                        in1=bias_tiles["v"].rearrange("p (h d) -> p h d", h=HPC),
                    )
                else:
                    nc.scalar.copy(out=Vaug[:, lkt, :, 0:64], in_=psr)

            for lqh in range(2):
                for mt in range(2):
                    emit_qk_proj("k", x_k, wk_sb, KT, mt, lqh)
            for lqh in range(2):
                for mt in range(2):
                    emit_qk_proj("q", x_q, wq_sb, QT, mt, lqh)

            # ---- global software-pipelined tick stream ----
            # block = (pr, qc); 16 lk ticks per block; 128 global ticks.
            o_tiles = {}  # block -> {hp: psum tile}
            tmp_tiles = {}  # quad -> tmp tile
            p_tiles = {}  # quad -> p tile

            def front(t):
                blk, k = t // NLKT, t % NLKT
                pr, qc = blk // 4, blk % 4
                j = t // 4
                g_sb = gp.tile([128, 2 * QC], BF16, tag="g")
                nc.sync.dma_start(out=g_sb, in_=gPK[pr, qc, k, :, :])
                if t % 4 == 0:
                    tmp_tiles[j] = tp.tile(
                        [128, 4, 2 * QC], BF16, tag="tmp", name=f"tmp{j}"
                    )
                tmp2 = tmp_tiles[j]
                s_w = ps_s.tile([128, 2 * QC], F32, tag="s", name=f"s_{t}")
                for hp in range(2):
                    nc.tensor.matmul(
                        s_w[:, hp * QC : (hp + 1) * QC],
                        lhsT=KT[
                            hp * 64 : hp * 64 + 64, pr, k * 128 : (k + 1) * 128
                        ],
                        rhs=QT[
                            hp * 64 : hp * 64 + 64, pr, qc * QC : (qc + 1) * QC
                        ],
                        start=True,
                        stop=True,
                    )
                dst = tmp2[:, t % 4, :]
                if OFF_MOD and t % OFF_MOD == OFF_PHASE:
                    sB = sb8.tile([128, 2 * QC], BF16, tag="sb")
                    nc.scalar.copy(out=sB, in_=s_w)
                    nc.gpsimd.tensor_mul(dst, sB, g_sb)
                else:
                    nc.vector.tensor_mul(out=dst, in0=s_w, in1=g_sb)
                if t % 4 == 3:
                    p2 = pp.tile([128, 4, 2 * QC], BF16, tag="p", name=f"p{j}")
                    nc.scalar.activation(
                        out=p2, in_=tmp_tiles.pop(j), func=AF.Exp,
                        bias=ebias, scale=1.0,
                    )
                    p_tiles[j] = p2
                    if masked:
                        for par in range(4):
                            tt = 4 * j + par
                            kk = tt % NLKT
                            qc2 = (tt // NLKT) % 4
                            mb_sb = gp.tile([128, QC], BF16, tag="mb")
                            nc.sync.dma_start(
                                out=mb_sb,
                                in_=mbT[
                                    kk * 128 : (kk + 1) * 128,
                                    qc2 * QC : (qc2 + 1) * QC,
                                ],
                            )
                            for hp in range(2):
                                nc.vector.tensor_mul(
                                    out=p2[:, par, hp * QC : (hp + 1) * QC],
                                    in0=p2[:, par, hp * QC : (hp + 1) * QC],
                                    in1=mb_sb,
                                )

            def back(t):
                blk, k = t // NLKT, t % NLKT
                pr, qc = blk // 4, blk % 4
                if k == 0:
                    o_tiles[blk] = {
                        hp: ps_o.tile(
                            [128, QC], F32, tag="o", name=f"o_{blk}_{hp}"
                        )
                        for hp in range(2)
                    }
                j = t // 4
                p2 = p_tiles[j]
                for hp in range(2):
                    nc.tensor.matmul(
                        o_tiles[blk][hp],
                        lhsT=Vaug[:, k, 2 * pr + hp, :],
                        rhs=p2[:, t % 4, hp * QC : (hp + 1) * QC],
                        start=(k == 0),
                        stop=(k == NLKT - 1),
                    )
                if t % 4 == 3:
                    p_tiles.pop(j)
                if k == NLKT - 1:
                    OUa = opl.tile([65, 2, QC], BF16, tag="ou")
                    for hp in range(2):
                        nc.scalar.copy(
                            out=OUa[:, hp, :], in_=o_tiles[blk][hp][0:65, :]
                        )
                    nc.gpsimd.dma_start(out=oU[pr, qc, :, :, :], in_=OUa)

            NG = 8 * NLKT
            for t in range(NG + PIPE):
                if t < NG:
                    front(t)
                if t < 8:  # v projections ride the first 8 ticks
                    emit_v_lkt(2 * t)
                    emit_v_lkt(2 * t + 1)
                if t >= PIPE:
                    back(t - PIPE)

    nc.finalize()
    return nc


def build_l2(use_bo: bool, use_gamma: bool, use_beta: bool):
    nc = bacc.Bacc("TRN2", target_bir_lowering=False)

    oTf = nc.declare_dram_parameter("oTf", [128, NKC, CH], BF16, isOutput=False)
    woTs = nc.declare_dram_parameter("woTs", [128, NKC, D], BF16, isOutput=False)
    qres = nc.declare_dram_parameter("qres", [4, 128, D], BF16, isOutput=False)
    if use_bo:
        boR = nc.declare_dram_parameter("boR", [1, D], F32, isOutput=False)
    if use_gamma:
        gaR = nc.declare_dram_parameter("gaR", [1, D], F32, isOutput=False)
    if use_beta:
        beR = nc.declare_dram_parameter("beR", [1, D], F32, isOutput=False)
    yout = nc.declare_dram_parameter("yout", [4, 128, D], BF16, isOutput=True)

    with tile.TileContext(nc) as tc:
        with (
            tc.tile_pool(name="ins", bufs=1) as ins,
            tc.tile_pool(name="res", bufs=4) as res,
            tc.tile_pool(name="xb", bufs=4) as xb,
            tc.tile_pool(name="st", bufs=4) as st,
            tc.tile_pool(name="ps", bufs=8, space="PSUM") as psp,
        ):
            oT_sb = ins.tile([128, NKC, CH], BF16, tag="ot")
            wo_sb = ins.tile([128, NKC, D], BF16, tag="wo")
            for kc in range(NKC):
                nc.sync.dma_start(out=oT_sb[:, kc, :], in_=oTf[:, kc, :])
                nc.sync.dma_start(out=wo_sb[:, kc, :], in_=woTs[:, kc, :])
            eps_sb = ins.tile([128, 1], F32, tag="eps")
            nc.vector.memset(eps_sb, EPS)
            bo_sb = ga_sb = be_sb = None
            if use_bo:
                bo_sb = ins.tile([128, D], F32, tag="bo")
                nc.sync.dma_start(out=bo_sb, in_=boR.ap().to_broadcast([128, D]))
            if use_gamma:
                ga_sb = ins.tile([128, D], F32, tag="ga")
                nc.sync.dma_start(out=ga_sb, in_=gaR.ap().to_broadcast([128, D]))
            if use_beta:
                be_sb = ins.tile([128, D], F32, tag="be")
                nc.sync.dma_start(out=be_sb, in_=beR.ap().to_broadcast([128, D]))

            q_tiles = []
            for m in range(4):
                q_sb = res.tile([128, D], BF16, tag="q", name=f"q{m}")
                nc.gpsimd.dma_start(out=q_sb, in_=qres[m, :, :])
                q_tiles.append(q_sb)

            warm = psp.tile([128, 512], F32, tag="mm", name="warm")
            for i in range(8):
                nc.tensor.matmul(
                    warm,
                    lhsT=wo_sb[:, 0, 0:128],
                    rhs=wo_sb[:, 0, 0:512],
                    start=True,
                    stop=True,
                    skip_group_check=True,
                )

            fused_ln = bo_sb is None
            ps_mn = {
                (m, n): psp.tile([128, 512], F32, tag="mm", name=f"mm{m}{n}")
                for m in range(4)
                for n in range(2)
            }
            # phase A: kc 0..6 for all (m, n) — streams behind the DMA
            for kc in range(NKC - 1):
                for m in range(4):
                    for n in range(2):
                        nc.tensor.matmul(
                            ps_mn[(m, n)],
                            lhsT=oT_sb[:, kc, m * 128 : (m + 1) * 128],
                            rhs=wo_sb[:, kc, n * 512 : (n + 1) * 512],
                            start=(kc == 0),
                            stop=False,
                        )

            # phase B: per m, final kc + LN chain (staggered tails)
            for m in range(4):
                for n in range(2):
                    nc.tensor.matmul(
                        ps_mn[(m, n)],
                        lhsT=oT_sb[:, NKC - 1, m * 128 : (m + 1) * 128],
                        rhs=wo_sb[:, NKC - 1, n * 512 : (n + 1) * 512],
                        start=False,
                        stop=True,
                    )
                q_sb = q_tiles[m]
                x = xb.tile([128, D], F32, tag="x")
                accs = st.tile([128, 2], F32, tag="accs")
                for n in range(2):
                    ps = ps_mn.pop((m, n))
                    if fused_ln:
                        nc.vector.scalar_tensor_tensor(
                            out=x[:, n * 512 : (n + 1) * 512],
                            in0=ps,
                            scalar=1.0,
                            in1=q_sb[:, n * 512 : (n + 1) * 512],
                            op0=mybir.AluOpType.mult,
                            op1=mybir.AluOpType.add,
                            accum_out=accs[:, n : n + 1],
                        )
                    else:
                        nc.vector.tensor_add(
                            out=x[:, n * 512 : (n + 1) * 512],
                            in0=ps,
                            in1=q_sb[:, n * 512 : (n + 1) * 512],
                        )
                if fused_ln:
                    scr = xb.tile([128, D], F32, tag="scr")
                    ssq = st.tile([128, 1], F32, tag="ssq")
                    nc.scalar.activation(
                        out=scr, in_=x, func=AF.Square, accum_out=ssq
                    )
                    mu = st.tile([128, 1], F32, tag="mu")
                    nc.vector.tensor_scalar(
                        out=mu,
                        in0=accs[:, 0:1],
                        scalar1=accs[:, 1:2],
                        scalar2=1.0 / D,
                        op0=mybir.AluOpType.add,
                        op1=mybir.AluOpType.mult,
                    )
                    musq = st.tile([128, 1], F32, tag="musq")
                    nc.vector.tensor_mul(out=musq, in0=mu, in1=mu)
                    var = st.tile([128, 1], F32, tag="var")
                    nc.vector.tensor_scalar(
                        out=var,
                        in0=ssq,
                        scalar1=1.0 / D,
                        scalar2=musq,
                        op0=mybir.AluOpType.mult,
                        op1=mybir.AluOpType.subtract,
                    )
                    std = st.tile([128, 1], F32, tag="std")
                    nc.scalar.activation(
                        out=std, in_=var, func=AF.Sqrt, bias=eps_sb, scale=1.0
                    )
                else:
                    if bo_sb is not None:
                        nc.vector.tensor_add(out=x, in0=x, in1=bo_sb)
                    stats = st.tile([128, 2, 6], F32, tag="stats")
                    for hh in range(2):
                        nc.vector.bn_stats(
                            out=stats[:, hh, :],
                            in_=x[:, hh * 512 : (hh + 1) * 512],
                        )
                    mv = st.tile([128, 2], F32, tag="mv")
                    nc.vector.bn_aggr(out=mv, in_=stats)
                    mu = mv[:, 0:1]
                    std = st.tile([128, 1], F32, tag="std")
                    nc.scalar.activation(
                        out=std, in_=mv[:, 1:2], func=AF.Sqrt, bias=eps_sb, scale=1.0
                    )
                rstd = st.tile([128, 1], F32, tag="rstd")
                nc.vector.reciprocal(out=rstd, in_=std)
                y = xb.tile([128, D], BF16, tag="y")
                nc.vector.tensor_scalar(
                    out=y,
                    in0=x,
                    scalar1=mu,
                    scalar2=rstd,
                    op0=mybir.AluOpType.subtract,
                    op1=mybir.AluOpType.mult,
                )
                if ga_sb is not None:
                    nc.vector.tensor_mul(out=y, in0=y, in1=ga_sb)
                if be_sb is not None:
                    nc.vector.tensor_add(out=y, in0=y, in1=be_sb)
                nc.gpsimd.dma_start(out=yout[m, :, :], in_=y)

    nc.finalize()
    return nc


_L1_CACHE = {}
_L2_CACHE = {}
LAST_RUNS = []  # (tag, nc, in_maps) of the most recent kernel() call, for profiling


def kernel(
    q, k, v, k_gate, mask, wq, bq, wk, bk, wv, bv, wo, bo, gamma, beta
):
    q = np.asarray(q, np.float32)
    k = np.asarray(k, np.float32)
    v = np.asarray(v, np.float32)
    k_gate = np.asarray(k_gate, np.float32)
    mask = np.asarray(mask)
    wq = np.asarray(wq, np.float32)
    wk = np.asarray(wk, np.float32)
    wv = np.asarray(wv, np.float32)
    wo = np.asarray(wo, np.float32)
    bq = np.asarray(bq, np.float32)
    bk = np.asarray(bk, np.float32)
    bv = np.asarray(bv, np.float32)
    bo = np.asarray(bo, np.float32)
    gamma = np.asarray(gamma, np.float32)
    beta = np.asarray(beta, np.float32)

    masked = bool(mask.any())
    use_bq = bool(np.any(bq))
    use_bk = bool(np.any(bk))
    use_bv = bool(np.any(bv))
    use_bo = bool(np.any(bo))
    use_gamma = bool(np.any(gamma != 1.0))
    use_beta = bool(np.any(beta))

    temp = float(np.float32(np.power(DK, 0.5)))

    key1 = (masked, use_bq, use_bk, use_bv)
    if key1 not in _L1_CACHE:
        _L1_CACHE[key1] = build_l1(*key1)
    nc1 = _L1_CACHE[key1]

    # ---- stage launch-1 inputs ----
    xT = {}
    for b in range(B):
        xT[("q", b)] = _bf(_kc_layout(q[b].T))
        xT[("k", b)] = _bf(_kc_layout(k[b].T))
        xT[("v", b)] = _bf(_kc_layout(v[b].T))
    wts = {}
    for hg in range(4):
        sl = slice(hg * MPC, (hg + 1) * MPC)
        wts[("q", hg)] = _bf(_kc_layout(wq[sl].T / temp))
        wts[("k", hg)] = _bf(_kc_layout(wk[sl].T))
        wts[("v", hg)] = _bf(_kc_layout(wv[sl].T))

    in_maps = []
    for c in range(NCORE):
        b, hg = c // 4, c % 4
        hsl = slice(hg * HPC, (hg + 1) * HPC)
        # gate pack: k_gate[b] is [head, lq, lk];
        # gPK[pr, qc, lkt, p, hp*512 + i] = g[2pr+hp, qc*512+i, lkt*128+p]
        gh = k_gate[b, hsl]  # [4, 2048, 2048]  (head, lq, lk)
        gr = gh.reshape(2, 2, 4, QC, NLKT, 128)  # pr, hp, qc, i, lkt, p
        gPK = _bf(gr.transpose(0, 2, 4, 5, 1, 3).reshape(2, 4, NLKT, 128, 2 * QC))
        m = {
            "qT": xT[("q", b)],
            "kT": xT[("k", b)],
            "vT": xT[("v", b)],
            "wqT": wts[("q", hg)],
            "wkT": wts[("k", hg)],
            "wvT": wts[("v", hg)],
            "gPK": gPK,
        }
        if use_bq:
            m["bqP"] = np.ascontiguousarray(
                (bq[hg * MPC : (hg + 1) * MPC] / temp).reshape(2, 128).T
            )
        if use_bk:
            m["bkP"] = np.ascontiguousarray(
                bk[hg * MPC : (hg + 1) * MPC].reshape(2, 128).T
            )
        if use_bv:
            m["bvR"] = bv[hg * MPC : (hg + 1) * MPC].reshape(1, MPC).copy()
        if masked:
            m["mbT"] = _bf((~mask[b]).astype(np.float32).T)
        in_maps.append(m)

    LAST_RUNS.clear()
    LAST_RUNS.append(("L1", nc1, in_maps))
    res1 = run_bass_kernel_spmd(nc1, in_maps, list(range(NCORE)))

    # assemble O_un^T per batch, normalize on host
    OTb = np.empty((B, H * DV, L), np.float32)
    DENb = np.empty((B, H, L), np.float32)
    for b in range(B):
        for hg in range(4):
            r = res1.results[b * 4 + hg]["oU"].astype(np.float32)
            # r: [pr, qc, 65, hp, QC]
            for pr in range(2):
                for hp in range(2):
                    h = hg * 4 + 2 * pr + hp
                    blk = r[pr, :, :, hp, :]  # [qc, 65, QC]
                    OTb[b, h * 64 : (h + 1) * 64, :] = np.concatenate(
                        [blk[qc, :64] for qc in range(4)], axis=1
                    )
                    DENb[b, h, :] = blk[:, 64, :].reshape(L)
    # normalize on host: rows h*64:(h+1)*64 divided by den[h]
    rd = 1.0 / DENb  # [B, H, L]
    OTb *= np.repeat(rd, DV, axis=1)

    key2 = (use_bo, use_gamma, use_beta)
    if key2 not in _L2_CACHE:
        _L2_CACHE[key2] = build_l2(*key2)
    nc2 = _L2_CACHE[key2]

    woTs = _bf(_kc_layout(wo.T))
    in_maps2 = []
    for c in range(NCORE):
        b, rchunk = c // 4, c % 4
        rows = slice(rchunk * CH, (rchunk + 1) * CH)
        otf = OTb[b][:, rows]  # [1024, 512] normalized
        m = {
            "oTf": _bf(otf.reshape(NKC, 128, CH).transpose(1, 0, 2)),
            "woTs": woTs,
            "qres": _bf(q[b, rows].reshape(4, 128, D)),
        }
        if use_bo:
            m["boR"] = bo.reshape(1, D).copy()
        if use_gamma:
            m["gaR"] = gamma.reshape(1, D).copy()
        if use_beta:
            m["beR"] = beta.reshape(1, D).copy()
        in_maps2.append(m)

    LAST_RUNS.append(("L2", nc2, in_maps2))
    res2 = run_bass_kernel_spmd(nc2, in_maps2, list(range(NCORE)))

    out = np.empty((B, L, D), np.float32)
    for c in range(NCORE):
        b, rchunk = c // 4, c % 4
        out[b, rchunk * CH : (rchunk + 1) * CH] = (
            res2.results[c]["yout"].astype(np.float32).reshape(CH, D)
        )
    return out


# revision 20
# speedup vs baseline: 1.1080x; 1.1080x over previous
"""Trainium2 Bass kernel for gated multi-head attention + residual + LayerNorm.

Problem (nn_CNP_5669356834854):
    B=2, L=2048, D=1024, H=16, DK=DV=64
    Q = q@wq.T+bq; K = k@wk.T+bk; V = v@wv.T+bv   (per-head split)
    attn = softmax((Q K^T / sqrt(DK)) * k_gate  [masked])
    out = LayerNorm(attn @ V @ wo.T + bo + q)

Sharding: 8 cores = (batch b) x (head-group hg, 4 heads).  L1 computes
UNNORMALIZED per-head attention outputs O^T plus softmax denominators
(ones-augmented V).  The host normalizes O (cheap elementwise) while
resharding; L2 shards (batch, 512-row chunk): output projection +
residual + LayerNorm.

L1 structure (all in transposed "T-space", no on-chip transposes):
  - 8 blocks = (pr in 2) x (qc in 4): each block covers 2 heads x 512 lq
    over 16 lk-tiles -> 128 global ticks.
  - per tick: S-tile [128, 2hp x 512] f32 in a 3-slot PSUM ring (6 banks)
    -> the DVE gate-multiply (one FD=1024 op/tick) streams back-to-back,
    fully decoupled from the PE.
  - exp on ACT over 4-tick quads (FD=4096).
  - O accumulation [128,512] per head = 2 PSUM banks total, block-serial.
  - software pipeline: back(t-PIPE) trails front(t) across block bounds.
  - every 8th tick's gate-multiply is offloaded (ACT copies S->bf16, the
    idle GpSimd engine multiplies) to balance DVE vs ACT.
  - dual DMA: Sync HWDGE queue (wk, x_k, x_v, gate slabs) in parallel with
    GpSimd SWDGE queue (wq, wv, x_q, outputs).
  - host-packed contiguous gate slabs (256KB per tick); PE warm-up matmuls.
"""

import numpy as np
import ml_dtypes

import concourse.bacc as bacc
import concourse.tile as tile
from concourse import mybir
from concourse.bass_utils import run_bass_kernel_spmd

B, L, D, H, DK, DV = 2, 2048, 1024, 16, 64, 64
EPS = 1e-5
NCORE = 8
HPC = 4  # heads per core
NKC = D // 128  # 8 contraction chunks
NLKT = 16  # lk tiles
CH = 512  # L2 row-chunk per core
MPC = HPC * DK  # 256 projected rows per core
QC = 512  # lq per block
EXP_BIAS = -20.0

F32 = mybir.dt.float32
BF16 = mybir.dt.bfloat16
NPBF16 = ml_dtypes.bfloat16
AF = mybir.ActivationFunctionType

PIPE = 12  # O-matmul lag in global ticks
OFF_MOD = 0  # offload every OFF_MODth tick to GpSimd (0 = off)
OFF_PHASE = 2
N_WARM = 12


def _bf(x):
    return np.ascontiguousarray(x).astype(NPBF16)


def _kc_layout(a):
    """[D, N] -> [128, NKC, N] with row r = kc*128+p  ->  [p, kc, :]."""
    d, n = a.shape
    assert d == NKC * 128
    return np.ascontiguousarray(a.reshape(NKC, 128, n).transpose(1, 0, 2))


def build_l1(masked: bool, use_bq: bool, use_bk: bool, use_bv: bool):
    nc = bacc.Bacc("TRN2", target_bir_lowering=False)

    qT = nc.declare_dram_parameter("qT", [128, NKC, L], BF16, isOutput=False)
    kT = nc.declare_dram_parameter("kT", [128, NKC, L], BF16, isOutput=False)
    vT = nc.declare_dram_parameter("vT", [128, NKC, L], BF16, isOutput=False)
    wqT = nc.declare_dram_parameter("wqT", [128, NKC, MPC], BF16, isOutput=False)
    wkT = nc.declare_dram_parameter("wkT", [128, NKC, MPC], BF16, isOutput=False)
    wvT = nc.declare_dram_parameter("wvT", [128, NKC, MPC], BF16, isOutput=False)
    # host-packed gate: gPK[pr, qc, lkt, p, hp*512 + i]
    gPK = nc.declare_dram_parameter(
        "gPK", [2, 4, NLKT, 128, 2 * QC], BF16, isOutput=False
    )
    if use_bq:
        bqP = nc.declare_dram_parameter("bqP", [128, 2], F32, isOutput=False)
    if use_bk:
        bkP = nc.declare_dram_parameter("bkP", [128, 2], F32, isOutput=False)
    if use_bv:
        bvR = nc.declare_dram_parameter("bvR", [1, MPC], F32, isOutput=False)
    if masked:
        mbT = nc.declare_dram_parameter("mbT", [L, L], BF16, isOutput=False)
    # unnormalized O (rows 0:64 per hp) + denominator (row 64)
    oU = nc.declare_dram_parameter("oU", [2, 4, 65, 2, QC], BF16, isOutput=True)

    with tile.TileContext(nc) as tc:
        with (
            tc.tile_pool(name="xs", bufs=3) as xs,
            tc.tile_pool(name="ws", bufs=1) as ws,
            tc.tile_pool(name="qk", bufs=1) as qk,
            tc.tile_pool(name="gp", bufs=4) as gp,
            tc.tile_pool(name="tp", bufs=2) as tp,
            tc.tile_pool(name="pp", bufs=4) as pp,
            tc.tile_pool(name="sb8", bufs=2) as sb8,
            tc.tile_pool(name="op", bufs=2) as opl,
            tc.tile_pool(name="ps_s", bufs=3, space="PSUM") as ps_s,
            tc.tile_pool(name="ps_o", bufs=2, space="PSUM") as ps_o,
        ):
            # ---- DMA streams ----
            # sync (HWDGE): wk, x_k, x_v, then gate slabs (emitted in fronts)
            wk_sb = ws.tile([128, NKC, MPC], BF16, tag="wk")
            nc.sync.dma_start(out=wk_sb, in_=wkT[:, :, :])
            x_k = xs.tile([128, NKC, L], BF16, tag="x", name="x_k")
            for kc in range(NKC):
                nc.sync.dma_start(out=x_k[:, kc, :], in_=kT[:, kc, :])
            x_v = xs.tile([128, NKC, L], BF16, tag="x", name="x_v")
            for kc in range(NKC):
                nc.sync.dma_start(out=x_v[:, kc, :], in_=vT[:, kc, :])
            # gpsimd (SWDGE): wq, wv, x_q, then oU outputs
            wq_sb = ws.tile([128, NKC, MPC], BF16, tag="wq")
            nc.gpsimd.dma_start(out=wq_sb, in_=wqT[:, :, :])
            wv_sb = ws.tile([128, NKC, MPC], BF16, tag="wv")
            nc.gpsimd.dma_start(out=wv_sb, in_=wvT[:, :, :])
            x_q = xs.tile([128, NKC, L], BF16, tag="x", name="x_q")
            for kc in range(NKC):
                nc.gpsimd.dma_start(out=x_q[:, kc, :], in_=qT[:, kc, :])

            QT = qk.tile([128, 2, L], BF16, tag="qt")
            KT = qk.tile([128, 2, L], BF16, tag="kt")
            Vaug = qk.tile([128, NLKT, HPC, 128], BF16, tag="va")
            nc.vector.memset(Vaug[:, :, :, 64:128], 1.0)
            ebias = ws.tile([128, 1], F32, tag="eb")
            nc.vector.memset(ebias, EXP_BIAS)

            bias_tiles = {}
            if use_bq:
                bq_sb = ws.tile([128, 2], F32, tag="bq")
                nc.sync.dma_start(out=bq_sb, in_=bqP[:, :])
                bias_tiles["q"] = bq_sb
            if use_bk:
                bk_sb = ws.tile([128, 2], F32, tag="bk")
                nc.sync.dma_start(out=bk_sb, in_=bkP[:, :])
                bias_tiles["k"] = bk_sb
            if use_bv:
                bv_sb = ws.tile([128, MPC], F32, tag="bv")
                nc.sync.dma_start(out=bv_sb, in_=bvR.ap().to_broadcast([128, MPC]))
                bias_tiles["v"] = bv_sb

            # ---- PE warm-up (on wq, discarded) ----
            warm = ps_s.tile([128, 2 * 512], F32, tag="s", name="warm")
            for i in range(N_WARM):
                nc.tensor.matmul(
                    warm[:, 0:MPC],
                    lhsT=wq_sb[:, 0, 0:128],
                    rhs=wq_sb[:, 0, :],
                    start=True,
                    stop=True,
                    skip_group_check=True,
                )

            # ---- projections ----
            def emit_qk_proj(name, x_sb, w_sb, dst, mt, lqh):
                ps = ps_s.tile(
                    [128, 2 * 512], F32, tag="s", name=f"pj_{name}{mt}{lqh}"
                )
                for c in range(2):
                    lo = lqh * 1024 + c * 512
                    for kc in range(NKC):
                        nc.tensor.matmul(
                            ps[:, c * 512 : (c + 1) * 512],
                            lhsT=w_sb[:, kc, mt * 128 : (mt + 1) * 128],
                            rhs=x_sb[:, kc, lo : lo + 512],
                            start=(kc == 0),
                            stop=(kc == NKC - 1),
                        )
                if name in bias_tiles:
                    nc.vector.tensor_scalar_add(
                        out=dst[:, mt, lqh * 1024 : (lqh + 1) * 1024],
                        in0=ps,
                        scalar1=bias_tiles[name][:, mt : mt + 1],
                    )
                else:
                    nc.scalar.copy(
                        out=dst[:, mt, lqh * 1024 : (lqh + 1) * 1024], in_=ps
                    )

            def emit_v_lkt(lkt):
                ps = ps_o.tile([128, MPC], F32, tag="o", name="pj_v")
                for kc in range(NKC):
                    nc.tensor.matmul(
                        ps,
                        lhsT=x_v[:, kc, lkt * 128 : (lkt + 1) * 128],
                        rhs=wv_sb[:, kc, :],
                        start=(kc == 0),
                        stop=(kc == NKC - 1),
                    )
                psr = ps.rearrange("p (h d) -> p h d", h=HPC)
                if "v" in bias_tiles:
                    nc.vector.tensor_add(
                        out=Vaug[:, lkt, :, 0:64],
                        in0=psr,
                        in1=bias_tiles["v"].rearrange("p (h d) -> p h d", h=HPC),
                    )
                else:
                    nc.scalar.copy(out=Vaug[:, lkt, :, 0:64], in_=psr)

            for lqh in range(2):
                for mt in range(2):
                    emit_qk_proj("k", x_k, wk_sb, KT, mt, lqh)
            for lqh in range(2):
                for mt in range(2):
                    emit_qk_proj("q", x_q, wq_sb, QT, mt, lqh)

            # ---- global software-pipelined tick stream ----
            # block = (pr, qc); 16 lk ticks per block; 128 global ticks.
            o_tiles = {}  # block -> {hp: psum tile}
            tmp_tiles = {}  # quad -> tmp tile
            p_tiles = {}  # quad -> p tile

            def front(t):
                blk, k = t // NLKT, t % NLKT
                pr, qc = blk // 4, blk % 4
                j = t // 4
                g_sb = gp.tile([128, 2 * QC], BF16, tag="g")
                nc.sync.dma_start(out=g_sb, in_=gPK[pr, qc, k, :, :])
                if t % 4 == 0:
                    tmp_tiles[j] = tp.tile(
                        [128, 4, 2 * QC], BF16, tag="tmp", name=f"tmp{j}"
                    )
                tmp2 = tmp_tiles[j]
                s_w = ps_s.tile([128, 2 * QC], F32, tag="s", name=f"s_{t}")
                for hp in range(2):
                    nc.tensor.matmul(
                        s_w[:, hp * QC : (hp + 1) * QC],
                        lhsT=KT[
                            hp * 64 : hp * 64 + 64, pr, k * 128 : (k + 1) * 128
                        ],
                        rhs=QT[
                            hp * 64 : hp * 64 + 64, pr, qc * QC : (qc + 1) * QC
                        ],
                        start=True,
                        stop=True,
                    )
                dst = tmp2[:, t % 4, :]
                if OFF_MOD and t % OFF_MOD == OFF_PHASE:
                    sB = sb8.tile([128, 2 * QC], BF16, tag="sb")
                    nc.scalar.copy(out=sB, in_=s_w)
                    nc.gpsimd.tensor_mul(dst, sB, g_sb)
                else:
                    nc.vector.tensor_mul(out=dst, in0=s_w, in1=g_sb)
                if t % 4 == 3:
                    p2 = pp.tile([128, 4, 2 * QC], BF16, tag="p", name=f"p{j}")
                    nc.scalar.activation(
                        out=p2, in_=tmp_tiles.pop(j), func=AF.Exp,
                        bias=ebias, scale=1.0,
                    )
                    p_tiles[j] = p2
                    if masked:
                        for par in range(4):
                            tt = 4 * j + par
                            kk = tt % NLKT
                            qc2 = (tt // NLKT) % 4
                            mb_sb = gp.tile([128, QC], BF16, tag="mb")
                            nc.sync.dma_start(
                                out=mb_sb,
                                in_=mbT[
                                    kk * 128 : (kk + 1) * 128,
                                    qc2 * QC : (qc2 + 1) * QC,
                                ],
                            )
                            for hp in range(2):
                                nc.vector.tensor_mul(
                                    out=p2[:, par, hp * QC : (hp + 1) * QC],
                                    in0=p2[:, par, hp * QC : (hp + 1) * QC],
                                    in1=mb_sb,
                                )

            def back(t):
                blk, k = t // NLKT, t % NLKT
                pr, qc = blk // 4, blk % 4
                if k == 0:
                    o_tiles[blk] = {
                        hp: ps_o.tile(
                            [128, QC], F32, tag="o", name=f"o_{blk}_{hp}"
                        )
                        for hp in range(2)
                    }
                j = t // 4
                p2 = p_tiles[j]
                for hp in range(2):
                    nc.tensor.matmul(
                        o_tiles[blk][hp],
                        lhsT=Vaug[:, k, 2 * pr + hp, :],
                        rhs=p2[:, t % 4, hp * QC : (hp + 1) * QC],
                        start=(k == 0),
                        stop=(k == NLKT - 1),
                    )
                if t % 4 == 3:
                    p_tiles.pop(j)
                if k == NLKT - 1:
                    OUa = opl.tile([65, 2, QC], BF16, tag="ou")
                    for hp in range(2):
                        nc.scalar.copy(
                            out=OUa[:, hp, :], in_=o_tiles[blk][hp][0:65, :]
                        )
                    nc.gpsimd.dma_start(out=oU[pr, qc, :, :, :], in_=OUa)

            NG = 8 * NLKT
            for t in range(NG + PIPE):
                if t >= PIPE:
                    back(t - PIPE)
                if t < NG:
                    front(t)
                if t < 8:  # v projections ride the first 8 ticks
                    emit_v_lkt(2 * t)
                    emit_v_lkt(2 * t + 1)

    nc.finalize()
    return nc


def build_l2(use_bo: bool, use_gamma: bool, use_beta: bool):
    nc = bacc.Bacc("TRN2", target_bir_lowering=False)

    oTf = nc.declare_dram_parameter("oTf", [128, NKC, CH], BF16, isOutput=False)
    woTs = nc.declare_dram_parameter("woTs", [128, NKC, D], BF16, isOutput=False)
    qres = nc.declare_dram_parameter("qres", [4, 128, D], BF16, isOutput=False)
    if use_bo:
        boR = nc.declare_dram_parameter("boR", [1, D], F32, isOutput=False)
    if use_gamma:
        gaR = nc.declare_dram_parameter("gaR", [1, D], F32, isOutput=False)
    if use_beta:
        beR = nc.declare_dram_parameter("beR", [1, D], F32, isOutput=False)
    yout = nc.declare_dram_parameter("yout", [4, 128, D], BF16, isOutput=True)

    with tile.TileContext(nc) as tc:
        with (
            tc.tile_pool(name="ins", bufs=1) as ins,
            tc.tile_pool(name="res", bufs=4) as res,
            tc.tile_pool(name="xb", bufs=4) as xb,
            tc.tile_pool(name="st", bufs=4) as st,
            tc.tile_pool(name="ps", bufs=8, space="PSUM") as psp,
        ):
            oT_sb = ins.tile([128, NKC, CH], BF16, tag="ot")
            wo_sb = ins.tile([128, NKC, D], BF16, tag="wo")
            for kc in range(NKC):
                nc.sync.dma_start(out=oT_sb[:, kc, :], in_=oTf[:, kc, :])
                nc.sync.dma_start(out=wo_sb[:, kc, :], in_=woTs[:, kc, :])
            eps_sb = ins.tile([128, 1], F32, tag="eps")
            nc.vector.memset(eps_sb, EPS)
            bo_sb = ga_sb = be_sb = None
            if use_bo:
                bo_sb = ins.tile([128, D], F32, tag="bo")
                nc.sync.dma_start(out=bo_sb, in_=boR.ap().to_broadcast([128, D]))
            if use_gamma:
                ga_sb = ins.tile([128, D], F32, tag="ga")
                nc.sync.dma_start(out=ga_sb, in_=gaR.ap().to_broadcast([128, D]))
            if use_beta:
                be_sb = ins.tile([128, D], F32, tag="be")
                nc.sync.dma_start(out=be_sb, in_=beR.ap().to_broadcast([128, D]))

            q_tiles = []
            for m in range(4):
                q_sb = res.tile([128, D], BF16, tag="q", name=f"q{m}")
                nc.gpsimd.dma_start(out=q_sb, in_=qres[m, :, :])
                q_tiles.append(q_sb)

            warm = psp.tile([128, 512], F32, tag="mm", name="warm")
            for i in range(8):
                nc.tensor.matmul(
                    warm,
                    lhsT=wo_sb[:, 0, 0:128],
                    rhs=wo_sb[:, 0, 0:512],
                    start=True,
                    stop=True,
                    skip_group_check=True,
                )

            fused_ln = bo_sb is None
            ps_mn = {
                (m, n): psp.tile([128, 512], F32, tag="mm", name=f"mm{m}{n}")
                for m in range(4)
                for n in range(2)
            }
            # phase A: kc 0..6 for all (m, n) — streams behind the DMA
            for kc in range(NKC - 1):
                for m in range(4):
                    for n in range(2):
                        nc.tensor.matmul(
                            ps_mn[(m, n)],
                            lhsT=oT_sb[:, kc, m * 128 : (m + 1) * 128],
                            rhs=wo_sb[:, kc, n * 512 : (n + 1) * 512],
                            start=(kc == 0),
                            stop=False,
                        )

            # phase B: per m, final kc + LN chain (staggered tails)
            for m in range(4):
                for n in range(2):
                    nc.tensor.matmul(
                        ps_mn[(m, n)],
                        lhsT=oT_sb[:, NKC - 1, m * 128 : (m + 1) * 128],
                        rhs=wo_sb[:, NKC - 1, n * 512 : (n + 1) * 512],
                        start=False,
                        stop=True,
                    )
                q_sb = q_tiles[m]
                x = xb.tile([128, D], F32, tag="x")
                accs = st.tile([128, 2], F32, tag="accs")
                for n in range(2):
                    ps = ps_mn.pop((m, n))
                    if fused_ln:
                        nc.vector.scalar_tensor_tensor(
                            out=x[:, n * 512 : (n + 1) * 512],
                            in0=ps,
                            scalar=1.0,
                            in1=q_sb[:, n * 512 : (n + 1) * 512],
                            op0=mybir.AluOpType.mult,
                            op1=mybir.AluOpType.add,
                            accum_out=accs[:, n : n + 1],
                        )
                    else:
                        nc.vector.tensor_add(
                            out=x[:, n * 512 : (n + 1) * 512],
                            in0=ps,
                            in1=q_sb[:, n * 512 : (n + 1) * 512],
                        )
                if fused_ln:
                    scr = xb.tile([128, D], F32, tag="scr")
                    ssq = st.tile([128, 1], F32, tag="ssq")
                    nc.scalar.activation(
                        out=scr, in_=x, func=AF.Square, accum_out=ssq
                    )
                    mu = st.tile([128, 1], F32, tag="mu")
                    nc.vector.tensor_scalar(
                        out=mu,
                        in0=accs[:, 0:1],
                        scalar1=accs[:, 1:2],
                        scalar2=1.0 / D,
                        op0=mybir.AluOpType.add,
                        op1=mybir.AluOpType.mult,
                    )
                    musq = st.tile([128, 1], F32, tag="musq")
                    nc.vector.tensor_mul(out=musq, in0=mu, in1=mu)
                    var = st.tile([128, 1], F32, tag="var")
                    nc.vector.tensor_scalar(
                        out=var,
                        in0=ssq,
                        scalar1=1.0 / D,
                        scalar2=musq,
                        op0=mybir.AluOpType.mult,
                        op1=mybir.AluOpType.subtract,
                    )
                    std = st.tile([128, 1], F32, tag="std")
                    nc.scalar.activation(
                        out=std, in_=var, func=AF.Sqrt, bias=eps_sb, scale=1.0
                    )
                else:
                    if bo_sb is not None:
                        nc.vector.tensor_add(out=x, in0=x, in1=bo_sb)
                    stats = st.tile([128, 2, 6], F32, tag="stats")
                    for hh in range(2):
                        nc.vector.bn_stats(
                            out=stats[:, hh, :],
                            in_=x[:, hh * 512 : (hh + 1) * 512],
                        )
                    mv = st.tile([128, 2], F32, tag="mv")
                    nc.vector.bn_aggr(out=mv, in_=stats)
                    mu = mv[:, 0:1]
                    std = st.tile([128, 1], F32, tag="std")
                    nc.scalar.activation(
                        out=std, in_=mv[:, 1:2], func=AF.Sqrt, bias=eps_sb, scale=1.0
                    )
                rstd = st.tile([128, 1], F32, tag="rstd")
                nc.vector.reciprocal(out=rstd, in_=std)
                y = xb.tile([128, D], BF16, tag="y")
                nc.vector.tensor_scalar(
                    out=y,
                    in0=x,
                    scalar1=mu,
                    scalar2=rstd,
                    op0=mybir.AluOpType.subtract,
                    op1=mybir.AluOpType.mult,
                )
                if ga_sb is not None:
                    nc.vector.tensor_mul(out=y, in0=y, in1=ga_sb)
                if be_sb is not None:
                    nc.vector.tensor_add(out=y, in0=y, in1=be_sb)
                nc.gpsimd.dma_start(out=yout[m, :, :], in_=y)

    nc.finalize()
    return nc


_L1_CACHE = {}
_L2_CACHE = {}
LAST_RUNS = []  # (tag, nc, in_maps) of the most recent kernel() call, for profiling


def kernel(
    q, k, v, k_gate, mask, wq, bq, wk, bk, wv, bv, wo, bo, gamma, beta
):
    q = np.asarray(q, np.float32)
    k = np.asarray(k, np.float32)
    v = np.asarray(v, np.float32)
    k_gate = np.asarray(k_gate, np.float32)
    mask = np.asarray(mask)
    wq = np.asarray(wq, np.float32)
    wk = np.asarray(wk, np.float32)
    wv = np.asarray(wv, np.float32)
    wo = np.asarray(wo, np.float32)
    bq = np.asarray(bq, np.float32)
    bk = np.asarray(bk, np.float32)
    bv = np.asarray(bv, np.float32)
    bo = np.asarray(bo, np.float32)
    gamma = np.asarray(gamma, np.float32)
    beta = np.asarray(beta, np.float32)

    masked = bool(mask.any())
    use_bq = bool(np.any(bq))
    use_bk = bool(np.any(bk))
    use_bv = bool(np.any(bv))
    use_bo = bool(np.any(bo))
    use_gamma = bool(np.any(gamma != 1.0))
    use_beta = bool(np.any(beta))

    temp = float(np.float32(np.power(DK, 0.5)))

    key1 = (masked, use_bq, use_bk, use_bv)
    if key1 not in _L1_CACHE:
        _L1_CACHE[key1] = build_l1(*key1)
    nc1 = _L1_CACHE[key1]

    # ---- stage launch-1 inputs ----
    xT = {}
    for b in range(B):
        xT[("q", b)] = _bf(_kc_layout(q[b].T))
        xT[("k", b)] = _bf(_kc_layout(k[b].T))
        xT[("v", b)] = _bf(_kc_layout(v[b].T))
    wts = {}
    for hg in range(4):
        sl = slice(hg * MPC, (hg + 1) * MPC)
        wts[("q", hg)] = _bf(_kc_layout(wq[sl].T / temp))
        wts[("k", hg)] = _bf(_kc_layout(wk[sl].T))
        wts[("v", hg)] = _bf(_kc_layout(wv[sl].T))

    in_maps = []
    for c in range(NCORE):
        b, hg = c // 4, c % 4
        hsl = slice(hg * HPC, (hg + 1) * HPC)
        # gate pack: k_gate[b] is [head, lq, lk];
        # gPK[pr, qc, lkt, p, hp*512 + i] = g[2pr+hp, qc*512+i, lkt*128+p]
        gh = k_gate[b, hsl]  # [4, 2048, 2048]  (head, lq, lk)
        gr = gh.reshape(2, 2, 4, QC, NLKT, 128)  # pr, hp, qc, i, lkt, p
        gPK = _bf(gr.transpose(0, 2, 4, 5, 1, 3).reshape(2, 4, NLKT, 128, 2 * QC))
        m = {
            "qT": xT[("q", b)],
            "kT": xT[("k", b)],
            "vT": xT[("v", b)],
            "wqT": wts[("q", hg)],
            "wkT": wts[("k", hg)],
            "wvT": wts[("v", hg)],
            "gPK": gPK,
        }
        if use_bq:
            m["bqP"] = np.ascontiguousarray(
                (bq[hg * MPC : (hg + 1) * MPC] / temp).reshape(2, 128).T
            )
        if use_bk:
            m["bkP"] = np.ascontiguousarray(
                bk[hg * MPC : (hg + 1) * MPC].reshape(2, 128).T
            )
        if use_bv:
            m["bvR"] = bv[hg * MPC : (hg + 1) * MPC].reshape(1, MPC).copy()
        if masked:
            m["mbT"] = _bf((~mask[b]).astype(np.float32).T)
        in_maps.append(m)

    LAST_RUNS.clear()
    LAST_RUNS.append(("L1", nc1, in_maps))
    res1 = run_bass_kernel_spmd(nc1, in_maps, list(range(NCORE)))

    # assemble O_un^T per batch, normalize on host
    OTb = np.empty((B, H * DV, L), np.float32)
    DENb = np.empty((B, H, L), np.float32)
    for b in range(B):
        for hg in range(4):
            r = res1.results[b * 4 + hg]["oU"].astype(np.float32)
            # r: [pr, qc, 65, hp, QC]
            for pr in range(2):
                for hp in range(2):
                    h = hg * 4 + 2 * pr + hp
                    blk = r[pr, :, :, hp, :]  # [qc, 65, QC]
                    OTb[b, h * 64 : (h + 1) * 64, :] = np.concatenate(
                        [blk[qc, :64] for qc in range(4)], axis=1
                    )
                    DENb[b, h, :] = blk[:, 64, :].reshape(L)
    # normalize on host: rows h*64:(h+1)*64 divided by den[h]
    rd = 1.0 / DENb  # [B, H, L]
    OTb *= np.repeat(rd, DV, axis=1)

    key2 = (use_bo, use_gamma, use_beta)
    if key2 not in _L2_CACHE:
        _L2_CACHE[key2] = build_l2(*key2)
    nc2 = _L2_CACHE[key2]

    woTs = _bf(_kc_layout(wo.T))
    in_maps2 = []
    for c in range(NCORE):
        b, rchunk = c // 4, c % 4
        rows = slice(rchunk * CH, (rchunk + 1) * CH)
        otf = OTb[b][:, rows]  # [1024, 512] normalized
        m = {
            "oTf": _bf(otf.reshape(NKC, 128, CH).transpose(1, 0, 2)),
            "woTs": woTs,
            "qres": _bf(q[b, rows].reshape(4, 128, D)),
        }
        if use_bo:
            m["boR"] = bo.reshape(1, D).copy()
        if use_gamma:
            m["gaR"] = gamma.reshape(1, D).copy()
        if use_beta:
            m["beR"] = beta.reshape(1, D).copy()
        in_maps2.append(m)

    LAST_RUNS.append(("L2", nc2, in_maps2))
    res2 = run_bass_kernel_spmd(nc2, in_maps2, list(range(NCORE)))

    out = np.empty((B, L, D), np.float32)
    for c in range(NCORE):
        b, rchunk = c // 4, c % 4
        out[b, rchunk * CH : (rchunk + 1) * CH] = (
            res2.results[c]["yout"].astype(np.float32).reshape(CH, D)
        )
    return out


# revision 25
# speedup vs baseline: 1.1258x; 1.0160x over previous
"""Trainium2 Bass kernel for gated multi-head attention + residual + LayerNorm.

Problem (nn_CNP_5669356834854):
    B=2, L=2048, D=1024, H=16, DK=DV=64
    Q = q@wq.T+bq; K = k@wk.T+bk; V = v@wv.T+bv   (per-head split)
    attn = softmax((Q K^T / sqrt(DK)) * k_gate  [masked])
    out = LayerNorm(attn @ V @ wo.T + bo + q)

Sharding: 8 cores = (batch b) x (head-group hg, 4 heads).  L1 computes
UNNORMALIZED per-head attention outputs O^T plus softmax denominators
(ones-augmented V).  The host normalizes O (cheap elementwise) while
resharding; L2 shards (batch, 512-row chunk): output projection +
residual + LayerNorm.

L1 structure (all in transposed "T-space", no on-chip transposes):
  - 8 blocks = (pr in 2) x (qc in 4): each block covers 2 heads x 512 lq
    over 16 lk-tiles -> 128 global ticks.
  - per tick: S-tile [128, 2hp x 512] f32 in a 3-slot PSUM ring (6 banks)
    -> the DVE gate-multiply (one FD=1024 op/tick) streams back-to-back,
    fully decoupled from the PE.
  - exp on ACT over 4-tick quads (FD=4096).
  - O accumulation [128,512] per head = 2 PSUM banks total, block-serial.
  - software pipeline: back(t-PIPE) trails front(t) across block bounds.
  - every 8th tick's gate-multiply is offloaded (ACT copies S->bf16, the
    idle GpSimd engine multiplies) to balance DVE vs ACT.
  - dual DMA: Sync HWDGE queue (wk, x_k, x_v, gate slabs) in parallel with
    GpSimd SWDGE queue (wq, wv, x_q, outputs).
  - host-packed contiguous gate slabs (256KB per tick); PE warm-up matmuls.
"""

import numpy as np
import ml_dtypes

import concourse.bacc as bacc
import concourse.tile as tile
from concourse import mybir
from concourse.bass_utils import run_bass_kernel_spmd

B, L, D, H, DK, DV = 2, 2048, 1024, 16, 64, 64
EPS = 1e-5
NCORE = 8
HPC = 4  # heads per core
NKC = D // 128  # 8 contraction chunks
NLKT = 16  # lk tiles
CH = 512  # L2 row-chunk per core
MPC = HPC * DK  # 256 projected rows per core
QC = 512  # lq per block
EXP_BIAS = -20.0

F32 = mybir.dt.float32
BF16 = mybir.dt.bfloat16
NPBF16 = ml_dtypes.bfloat16
AF = mybir.ActivationFunctionType

PIPE = 12  # O-matmul lag in global ticks
OFF_MOD = 0  # offload every OFF_MODth tick to GpSimd (0 = off)
OFF_PHASE = 2
N_WARM = 12


def _bf(x):
    return np.ascontiguousarray(x).astype(NPBF16)


def _kc_layout(a):
    """[D, N] -> [128, NKC, N] with row r = kc*128+p  ->  [p, kc, :]."""
    d, n = a.shape
    assert d == NKC * 128
    return np.ascontiguousarray(a.reshape(NKC, 128, n).transpose(1, 0, 2))


def build_l1(masked: bool, use_bq: bool, use_bk: bool, use_bv: bool):
    nc = bacc.Bacc("TRN2", target_bir_lowering=False)

    qT = nc.declare_dram_parameter("qT", [128, NKC, L], BF16, isOutput=False)
    kT = nc.declare_dram_parameter("kT", [128, NKC, L], BF16, isOutput=False)
    vT = nc.declare_dram_parameter("vT", [128, NKC, L], BF16, isOutput=False)
    wqT = nc.declare_dram_parameter("wqT", [128, NKC, MPC], BF16, isOutput=False)
    wkT = nc.declare_dram_parameter("wkT", [128, NKC, MPC], BF16, isOutput=False)
    wvT = nc.declare_dram_parameter("wvT", [128, NKC, MPC], BF16, isOutput=False)
    # host-packed gate: gPK[pr, qc, lkt, p, hp*512 + i]
    gPK = nc.declare_dram_parameter(
        "gPK", [2, 4, NLKT, 128, 2 * QC], BF16, isOutput=False
    )
    if use_bq:
        bqP = nc.declare_dram_parameter("bqP", [128, 2], F32, isOutput=False)
    if use_bk:
        bkP = nc.declare_dram_parameter("bkP", [128, 2], F32, isOutput=False)
    if use_bv:
        bvR = nc.declare_dram_parameter("bvR", [1, MPC], F32, isOutput=False)
    if masked:
        mbT = nc.declare_dram_parameter("mbT", [L, L], BF16, isOutput=False)
    # unnormalized O (rows 0:64 per hp) + denominator (row 64)
    oU = nc.declare_dram_parameter("oU", [2, 4, 65, 2, QC], BF16, isOutput=True)

    with tile.TileContext(nc) as tc:
        with (
            tc.tile_pool(name="xs", bufs=3) as xs,
            tc.tile_pool(name="ws", bufs=1) as ws,
            tc.tile_pool(name="qk", bufs=1) as qk,
            tc.tile_pool(name="gp", bufs=4) as gp,
            tc.tile_pool(name="tp", bufs=2) as tp,
            tc.tile_pool(name="pp", bufs=4) as pp,
            tc.tile_pool(name="sb8", bufs=2) as sb8,
            tc.tile_pool(name="op", bufs=2) as opl,
            tc.tile_pool(name="ps_s", bufs=3, space="PSUM") as ps_s,
            tc.tile_pool(name="ps_o", bufs=2, space="PSUM") as ps_o,
        ):
            # ---- DMA streams ----
            # sync (HWDGE): wk, x_k, x_v, then gate slabs (emitted in fronts)
            wk_sb = ws.tile([128, NKC, MPC], BF16, tag="wk")
            nc.sync.dma_start(out=wk_sb, in_=wkT[:, :, :])
            x_k = xs.tile([128, NKC, L], BF16, tag="x", name="x_k")
            for kc in range(NKC):
                nc.sync.dma_start(out=x_k[:, kc, :], in_=kT[:, kc, :])
            x_q = xs.tile([128, NKC, L], BF16, tag="x", name="x_q")
            for kc in range(NKC):
                nc.sync.dma_start(out=x_q[:, kc, :], in_=qT[:, kc, :])
            # first gate slabs jump ahead of x_v in the sync queue
            G_PRE = 4
            g_pre = {}
            for t0 in range(G_PRE):
                gt = gp.tile([128, 2 * QC], BF16, tag="g", name=f"gpre{t0}")
                nc.sync.dma_start(out=gt, in_=gPK[0, 0, t0, :, :])
                g_pre[t0] = gt
            x_v = xs.tile([128, NKC, L], BF16, tag="x", name="x_v")
            for kc in range(NKC):
                nc.sync.dma_start(out=x_v[:, kc, :], in_=vT[:, kc, :])
            # gpsimd (SWDGE): wq, wv, then oU outputs
            wq_sb = ws.tile([128, NKC, MPC], BF16, tag="wq")
            nc.gpsimd.dma_start(out=wq_sb, in_=wqT[:, :, :])
            wv_sb = ws.tile([128, NKC, MPC], BF16, tag="wv")
            nc.gpsimd.dma_start(out=wv_sb, in_=wvT[:, :, :])

            QT = qk.tile([128, 2, L], BF16, tag="qt")
            KT = qk.tile([128, 2, L], BF16, tag="kt")
            Vaug = qk.tile([128, NLKT, HPC, 128], BF16, tag="va")
            nc.vector.memset(Vaug[:, :, :, 64:128], 1.0)
            ebias = ws.tile([128, 1], F32, tag="eb")
            nc.vector.memset(ebias, EXP_BIAS)

            bias_tiles = {}
            if use_bq:
                bq_sb = ws.tile([128, 2], F32, tag="bq")
                nc.sync.dma_start(out=bq_sb, in_=bqP[:, :])
                bias_tiles["q"] = bq_sb
            if use_bk:
                bk_sb = ws.tile([128, 2], F32, tag="bk")
                nc.sync.dma_start(out=bk_sb, in_=bkP[:, :])
                bias_tiles["k"] = bk_sb
            if use_bv:
                bv_sb = ws.tile([128, MPC], F32, tag="bv")
                nc.sync.dma_start(out=bv_sb, in_=bvR.ap().to_broadcast([128, MPC]))
                bias_tiles["v"] = bv_sb

            # ---- PE warm-up (on wq, discarded) ----
            warm = ps_s.tile([128, 2 * 512], F32, tag="s", name="warm")
            for i in range(N_WARM):
                nc.tensor.matmul(
                    warm[:, 0:MPC],
                    lhsT=wq_sb[:, 0, 0:128],
                    rhs=wq_sb[:, 0, :],
                    start=True,
                    stop=True,
                    skip_group_check=True,
                )

            # ---- projections ----
            def emit_qk_proj(name, x_sb, w_sb, dst, mt, lqh):
                ps = ps_s.tile(
                    [128, 2 * 512], F32, tag="s", name=f"pj_{name}{mt}{lqh}"
                )
                for c in range(2):
                    lo = lqh * 1024 + c * 512
                    for kc in range(NKC):
                        nc.tensor.matmul(
                            ps[:, c * 512 : (c + 1) * 512],
                            lhsT=w_sb[:, kc, mt * 128 : (mt + 1) * 128],
                            rhs=x_sb[:, kc, lo : lo + 512],
                            start=(kc == 0),
                            stop=(kc == NKC - 1),
                        )
                if name in bias_tiles:
                    nc.vector.tensor_scalar_add(
                        out=dst[:, mt, lqh * 1024 : (lqh + 1) * 1024],
                        in0=ps,
                        scalar1=bias_tiles[name][:, mt : mt + 1],
                    )
                else:
                    nc.scalar.copy(
                        out=dst[:, mt, lqh * 1024 : (lqh + 1) * 1024], in_=ps
                    )

            def emit_v_lkt(lkt):
                ps = ps_o.tile([128, MPC], F32, tag="o", name="pj_v")
                for kc in range(NKC):
                    nc.tensor.matmul(
                        ps,
                        lhsT=x_v[:, kc, lkt * 128 : (lkt + 1) * 128],
                        rhs=wv_sb[:, kc, :],
                        start=(kc == 0),
                        stop=(kc == NKC - 1),
                    )
                psr = ps.rearrange("p (h d) -> p h d", h=HPC)
                if "v" in bias_tiles:
                    nc.vector.tensor_add(
                        out=Vaug[:, lkt, :, 0:64],
                        in0=psr,
                        in1=bias_tiles["v"].rearrange("p (h d) -> p h d", h=HPC),
                    )
                else:
                    nc.scalar.copy(out=Vaug[:, lkt, :, 0:64], in_=psr)

            for lqh in range(2):
                for mt in range(2):
                    emit_qk_proj("k", x_k, wk_sb, KT, mt, lqh)
            for lqh in range(2):
                for mt in range(2):
                    emit_qk_proj("q", x_q, wq_sb, QT, mt, lqh)

            # ---- global software-pipelined tick stream ----
            # block = (pr, qc); 16 lk ticks per block; 128 global ticks.
            o_tiles = {}  # block -> {hp: psum tile}
            tmp_tiles = {}  # quad -> tmp tile
            p_tiles = {}  # quad -> p tile

            def front(t):
                blk, k = t // NLKT, t % NLKT
                pr, qc = blk // 4, blk % 4
                j = t // 4
                if t in g_pre:
                    g_sb = g_pre.pop(t)
                else:
                    g_sb = gp.tile([128, 2 * QC], BF16, tag="g")
                    nc.sync.dma_start(out=g_sb, in_=gPK[pr, qc, k, :, :])
                if t % 4 == 0:
                    tmp_tiles[j] = tp.tile(
                        [128, 4, 2 * QC], BF16, tag="tmp", name=f"tmp{j}"
                    )
                tmp2 = tmp_tiles[j]
                s_w = ps_s.tile([128, 2 * QC], F32, tag="s", name=f"s_{t}")
                for hp in range(2):
                    nc.tensor.matmul(
                        s_w[:, hp * QC : (hp + 1) * QC],
                        lhsT=KT[
                            hp * 64 : hp * 64 + 64, pr, k * 128 : (k + 1) * 128
                        ],
                        rhs=QT[
                            hp * 64 : hp * 64 + 64, pr, qc * QC : (qc + 1) * QC
                        ],
                        start=True,
                        stop=True,
                    )
                dst = tmp2[:, t % 4, :]
                if OFF_MOD and t % OFF_MOD == OFF_PHASE:
                    sB = sb8.tile([128, 2 * QC], BF16, tag="sb")
                    nc.scalar.copy(out=sB, in_=s_w)
                    nc.gpsimd.tensor_mul(dst, sB, g_sb)
                else:
                    nc.vector.tensor_mul(out=dst, in0=s_w, in1=g_sb)
                if t % 4 == 3:
                    p2 = pp.tile([128, 4, 2 * QC], BF16, tag="p", name=f"p{j}")
                    nc.scalar.activation(
                        out=p2, in_=tmp_tiles.pop(j), func=AF.Exp,
                        bias=ebias, scale=1.0,
                    )
                    p_tiles[j] = p2
                    if masked:
                        for par in range(4):
                            tt = 4 * j + par
                            kk = tt % NLKT
                            qc2 = (tt // NLKT) % 4
                            mb_sb = gp.tile([128, QC], BF16, tag="mb")
                            nc.sync.dma_start(
                                out=mb_sb,
                                in_=mbT[
                                    kk * 128 : (kk + 1) * 128,
                                    qc2 * QC : (qc2 + 1) * QC,
                                ],
                            )
                            for hp in range(2):
                                nc.vector.tensor_mul(
                                    out=p2[:, par, hp * QC : (hp + 1) * QC],
                                    in0=p2[:, par, hp * QC : (hp + 1) * QC],
                                    in1=mb_sb,
                                )

            def back(t):
                blk, k = t // NLKT, t % NLKT
                pr, qc = blk // 4, blk % 4
                if k == 0:
                    o_tiles[blk] = {
                        hp: ps_o.tile(
                            [128, QC], F32, tag="o", name=f"o_{blk}_{hp}"
                        )
                        for hp in range(2)
                    }
                j = t // 4
                p2 = p_tiles[j]
                for hp in range(2):
                    nc.tensor.matmul(
                        o_tiles[blk][hp],
                        lhsT=Vaug[:, k, 2 * pr + hp, :],
                        rhs=p2[:, t % 4, hp * QC : (hp + 1) * QC],
                        start=(k == 0),
                        stop=(k == NLKT - 1),
                    )
                if t % 4 == 3:
                    p_tiles.pop(j)
                if k == NLKT - 1:
                    OUa = opl.tile([65, 2, QC], BF16, tag="ou")
                    for hp in range(2):
                        nc.scalar.copy(
                            out=OUa[:, hp, :], in_=o_tiles[blk][hp][0:65, :]
                        )
                    nc.gpsimd.dma_start(out=oU[pr, qc, :, :, :], in_=OUa)

            NG = 8 * NLKT
            for t in range(NG + PIPE):
                if t >= PIPE:
                    back(t - PIPE)
                if t < NG:
                    front(t)
                if 8 <= t < 12:  # all v psums must clear the "o" ring
                    for i in range(4):  # before back(0) claims it at t=12
                        emit_v_lkt(4 * (t - 8) + i)

    nc.finalize()
    return nc


def build_l2(use_bo: bool, use_gamma: bool, use_beta: bool):
    nc = bacc.Bacc("TRN2", target_bir_lowering=False)

    oTf = nc.declare_dram_parameter("oTf", [128, NKC, CH], BF16, isOutput=False)
    woTs = nc.declare_dram_parameter("woTs", [128, NKC, D], BF16, isOutput=False)
    qres = nc.declare_dram_parameter("qres", [4, 128, D], BF16, isOutput=False)
    if use_bo:
        boR = nc.declare_dram_parameter("boR", [1, D], F32, isOutput=False)
    if use_gamma:
        gaR = nc.declare_dram_parameter("gaR", [1, D], F32, isOutput=False)
    if use_beta:
        beR = nc.declare_dram_parameter("beR", [1, D], F32, isOutput=False)
    yout = nc.declare_dram_parameter("yout", [4, 128, D], BF16, isOutput=True)

    with tile.TileContext(nc) as tc:
        with (
            tc.tile_pool(name="ins", bufs=1) as ins,
            tc.tile_pool(name="res", bufs=4) as res,
            tc.tile_pool(name="xb", bufs=4) as xb,
            tc.tile_pool(name="st", bufs=4) as st,
            tc.tile_pool(name="ps", bufs=8, space="PSUM") as psp,
        ):
            oT_sb = ins.tile([128, NKC, CH], BF16, tag="ot")
            wo_sb = ins.tile([128, NKC, D], BF16, tag="wo")
            for kc in range(NKC):
                nc.sync.dma_start(out=oT_sb[:, kc, :], in_=oTf[:, kc, :])
                nc.sync.dma_start(out=wo_sb[:, kc, :], in_=woTs[:, kc, :])
            eps_sb = ins.tile([128, 1], F32, tag="eps")
            nc.vector.memset(eps_sb, EPS)
            bo_sb = ga_sb = be_sb = None
            if use_bo:
                bo_sb = ins.tile([128, D], F32, tag="bo")
                nc.sync.dma_start(out=bo_sb, in_=boR.ap().to_broadcast([128, D]))
            if use_gamma:
                ga_sb = ins.tile([128, D], F32, tag="ga")
                nc.sync.dma_start(out=ga_sb, in_=gaR.ap().to_broadcast([128, D]))
            if use_beta:
                be_sb = ins.tile([128, D], F32, tag="be")
                nc.sync.dma_start(out=be_sb, in_=beR.ap().to_broadcast([128, D]))

            q_tiles = []
            for m in range(4):
                q_sb = res.tile([128, D], BF16, tag="q", name=f"q{m}")
                nc.gpsimd.dma_start(out=q_sb, in_=qres[m, :, :])
                q_tiles.append(q_sb)

            warm = psp.tile([128, 512], F32, tag="mm", name="warm")
            for i in range(8):
                nc.tensor.matmul(
                    warm,
                    lhsT=wo_sb[:, 0, 0:128],
                    rhs=wo_sb[:, 0, 0:512],
                    start=True,
                    stop=True,
                    skip_group_check=True,
                )

            fused_ln = bo_sb is None
            ps_mn = {
                (m, n): psp.tile([128, 512], F32, tag="mm", name=f"mm{m}{n}")
                for m in range(4)
                for n in range(2)
            }
            # phase A: kc 0..6 for all (m, n) — streams behind the DMA
            for kc in range(NKC - 1):
                for m in range(4):
                    for n in range(2):
                        nc.tensor.matmul(
                            ps_mn[(m, n)],
                            lhsT=oT_sb[:, kc, m * 128 : (m + 1) * 128],
                            rhs=wo_sb[:, kc, n * 512 : (n + 1) * 512],
                            start=(kc == 0),
                            stop=False,
                        )

            # phase B: per m, final kc + LN chain (staggered tails)
            for m in range(4):
                for n in range(2):
                    nc.tensor.matmul(
                        ps_mn[(m, n)],
                        lhsT=oT_sb[:, NKC - 1, m * 128 : (m + 1) * 128],
                        rhs=wo_sb[:, NKC - 1, n * 512 : (n + 1) * 512],
                        start=False,
                        stop=True,
                    )
                q_sb = q_tiles[m]
                x = xb.tile([128, D], F32, tag="x")
                accs = st.tile([128, 2], F32, tag="accs")
                for n in range(2):
                    ps = ps_mn.pop((m, n))
                    if fused_ln:
                        nc.vector.scalar_tensor_tensor(
                            out=x[:, n * 512 : (n + 1) * 512],
                            in0=ps,
                            scalar=1.0,
                            in1=q_sb[:, n * 512 : (n + 1) * 512],
                            op0=mybir.AluOpType.mult,
                            op1=mybir.AluOpType.add,
                            accum_out=accs[:, n : n + 1],
                        )
                    else:
                        nc.vector.tensor_add(
                            out=x[:, n * 512 : (n + 1) * 512],
                            in0=ps,
                            in1=q_sb[:, n * 512 : (n + 1) * 512],
                        )
                if fused_ln:
                    scr = xb.tile([128, D], F32, tag="scr")
                    ssq = st.tile([128, 1], F32, tag="ssq")
                    nc.scalar.activation(
                        out=scr, in_=x, func=AF.Square, accum_out=ssq
                    )
                    mu = st.tile([128, 1], F32, tag="mu")
                    nc.vector.tensor_scalar(
                        out=mu,
                        in0=accs[:, 0:1],
                        scalar1=accs[:, 1:2],
                        scalar2=1.0 / D,
                        op0=mybir.AluOpType.add,
                        op1=mybir.AluOpType.mult,
                    )
                    musq = st.tile([128, 1], F32, tag="musq")
                    nc.vector.tensor_mul(out=musq, in0=mu, in1=mu)
                    var = st.tile([128, 1], F32, tag="var")
                    nc.vector.tensor_scalar(
                        out=var,
                        in0=ssq,
                        scalar1=1.0 / D,
                        scalar2=musq,
                        op0=mybir.AluOpType.mult,
                        op1=mybir.AluOpType.subtract,
                    )
                    std = st.tile([128, 1], F32, tag="std")
                    nc.scalar.activation(
                        out=std, in_=var, func=AF.Sqrt, bias=eps_sb, scale=1.0
                    )
                else:
                    if bo_sb is not None:
                        nc.vector.tensor_add(out=x, in0=x, in1=bo_sb)
                    stats = st.tile([128, 2, 6], F32, tag="stats")
                    for hh in range(2):
                        nc.vector.bn_stats(
                            out=stats[:, hh, :],
                            in_=x[:, hh * 512 : (hh + 1) * 512],
                        )
                    mv = st.tile([128, 2], F32, tag="mv")
                    nc.vector.bn_aggr(out=mv, in_=stats)
                    mu = mv[:, 0:1]
                    std = st.tile([128, 1], F32, tag="std")
                    nc.scalar.activation(
                        out=std, in_=mv[:, 1:2], func=AF.Sqrt, bias=eps_sb, scale=1.0
                    )
                rstd = st.tile([128, 1], F32, tag="rstd")
                nc.vector.reciprocal(out=rstd, in_=std)
                y = xb.tile([128, D], BF16, tag="y")
                nc.vector.tensor_scalar(
                    out=y,
                    in0=x,
                    scalar1=mu,
                    scalar2=rstd,
                    op0=mybir.AluOpType.subtract,
                    op1=mybir.AluOpType.mult,
                )
                if ga_sb is not None:
                    nc.vector.tensor_mul(out=y, in0=y, in1=ga_sb)
                if be_sb is not None:
                    nc.vector.tensor_add(out=y, in0=y, in1=be_sb)
                nc.gpsimd.dma_start(out=yout[m, :, :], in_=y)

    nc.finalize()
    return nc


_L1_CACHE = {}
_L2_CACHE = {}
LAST_RUNS = []  # (tag, nc, in_maps) of the most recent kernel() call, for profiling


def kernel(
    q, k, v, k_gate, mask, wq, bq, wk, bk, wv, bv, wo, bo, gamma, beta
):
    q = np.asarray(q, np.float32)
    k = np.asarray(k, np.float32)
    v = np.asarray(v, np.float32)
    k_gate = np.asarray(k_gate, np.float32)
    mask = np.asarray(mask)
    wq = np.asarray(wq, np.float32)
    wk = np.asarray(wk, np.float32)
    wv = np.asarray(wv, np.float32)
    wo = np.asarray(wo, np.float32)
    bq = np.asarray(bq, np.float32)
    bk = np.asarray(bk, np.float32)
    bv = np.asarray(bv, np.float32)
    bo = np.asarray(bo, np.float32)
    gamma = np.asarray(gamma, np.float32)
    beta = np.asarray(beta, np.float32)

    masked = bool(mask.any())
    use_bq = bool(np.any(bq))
    use_bk = bool(np.any(bk))
    use_bv = bool(np.any(bv))
    use_bo = bool(np.any(bo))
    use_gamma = bool(np.any(gamma != 1.0))
    use_beta = bool(np.any(beta))

    temp = float(np.float32(np.power(DK, 0.5)))

    key1 = (masked, use_bq, use_bk, use_bv)
    if key1 not in _L1_CACHE:
        _L1_CACHE[key1] = build_l1(*key1)
    nc1 = _L1_CACHE[key1]

    # ---- stage launch-1 inputs ----
    xT = {}
    for b in range(B):
        xT[("q", b)] = _bf(_kc_layout(q[b].T))
        xT[("k", b)] = _bf(_kc_layout(k[b].T))
        xT[("v", b)] = _bf(_kc_layout(v[b].T))
    wts = {}
    for hg in range(4):
        sl = slice(hg * MPC, (hg + 1) * MPC)
        wts[("q", hg)] = _bf(_kc_layout(wq[sl].T / temp))
        wts[("k", hg)] = _bf(_kc_layout(wk[sl].T))
        wts[("v", hg)] = _bf(_kc_layout(wv[sl].T))

    in_maps = []
    for c in range(NCORE):
        b, hg = c // 4, c % 4
        hsl = slice(hg * HPC, (hg + 1) * HPC)
        # gate pack: k_gate[b] is [head, lq, lk];
        # gPK[pr, qc, lkt, p, hp*512 + i] = g[2pr+hp, qc*512+i, lkt*128+p]
        gh = k_gate[b, hsl]  # [4, 2048, 2048]  (head, lq, lk)
        gr = gh.reshape(2, 2, 4, QC, NLKT, 128)  # pr, hp, qc, i, lkt, p
        gPK = _bf(gr.transpose(0, 2, 4, 5, 1, 3).reshape(2, 4, NLKT, 128, 2 * QC))
        m = {
            "qT": xT[("q", b)],
            "kT": xT[("k", b)],
            "vT": xT[("v", b)],
            "wqT": wts[("q", hg)],
            "wkT": wts[("k", hg)],
            "wvT": wts[("v", hg)],
            "gPK": gPK,
        }
        if use_bq:
            m["bqP"] = np.ascontiguousarray(
                (bq[hg * MPC : (hg + 1) * MPC] / temp).reshape(2, 128).T
            )
        if use_bk:
            m["bkP"] = np.ascontiguousarray(
                bk[hg * MPC : (hg + 1) * MPC].reshape(2, 128).T
            )
        if use_bv:
            m["bvR"] = bv[hg * MPC : (hg + 1) * MPC].reshape(1, MPC).copy()
        if masked:
            m["mbT"] = _bf((~mask[b]).astype(np.float32).T)
        in_maps.append(m)

    LAST_RUNS.clear()
    LAST_RUNS.append(("L1", nc1, in_maps))
    res1 = run_bass_kernel_spmd(nc1, in_maps, list(range(NCORE)))

    # assemble O_un^T per batch, normalize on host
    OTb = np.empty((B, H * DV, L), np.float32)
    DENb = np.empty((B, H, L), np.float32)
    for b in range(B):
        for hg in range(4):
            r = res1.results[b * 4 + hg]["oU"].astype(np.float32)
            # r: [pr, qc, 65, hp, QC]
            for pr in range(2):
                for hp in range(2):
                    h = hg * 4 + 2 * pr + hp
                    blk = r[pr, :, :, hp, :]  # [qc, 65, QC]
                    OTb[b, h * 64 : (h + 1) * 64, :] = np.concatenate(
                        [blk[qc, :64] for qc in range(4)], axis=1
                    )
                    DENb[b, h, :] = blk[:, 64, :].reshape(L)
    # normalize on host: rows h*64:(h+1)*64 divided by den[h]
    rd = 1.0 / DENb  # [B, H, L]
    OTb *= np.repeat(rd, DV, axis=1)

    key2 = (use_bo, use_gamma, use_beta)
    if key2 not in _L2_CACHE:
        _L2_CACHE[key2] = build_l2(*key2)
    nc2 = _L2_CACHE[key2]

    woTs = _bf(_kc_layout(wo.T))
    in_maps2 = []
    for c in range(NCORE):
        b, rchunk = c // 4, c % 4
        rows = slice(rchunk * CH, (rchunk + 1) * CH)
        otf = OTb[b][:, rows]  # [1024, 512] normalized
        m = {
            "oTf": _bf(otf.reshape(NKC, 128, CH).transpose(1, 0, 2)),
            "woTs": woTs,
            "qres": _bf(q[b, rows].reshape(4, 128, D)),
        }
        if use_bo:
            m["boR"] = bo.reshape(1, D).copy()
        if use_gamma:
            m["gaR"] = gamma.reshape(1, D).copy()
        if use_beta:
            m["beR"] = beta.reshape(1, D).copy()
        in_maps2.append(m)

    LAST_RUNS.append(("L2", nc2, in_maps2))
    res2 = run_bass_kernel_spmd(nc2, in_maps2, list(range(NCORE)))

    out = np.empty((B, L, D), np.float32)
    for c in range(NCORE):
        b, rchunk = c // 4, c % 4
        out[b, rchunk * CH : (rchunk + 1) * CH] = (
            res2.results[c]["yout"].astype(np.float32).reshape(CH, D)
        )
    return out
